# revision 56
# baseline (speedup 1.0000x reference)
"""AttentionMILPooling Trainium2 kernel (hybrid xt/xb design).

Math (matches the jax reference):
    scores  = tanh(X @ W1 + b1) @ W2 + b2          # [T, 1]
    weights = softmax(scores, axis=0)              # global over all T
    out[b]  = sum_{i in bag b} weights[i] * X[i]   # [64, 512]

Identities:
  * b2 cancels in the softmax -> dropped.
  * |scores| <= sum|W2| ~ 13, exp fits fp32/bf16 range -> no max-subtract.
  * out[b] = U[b] / Z with U[b] = sum_{i in b} exp(s_i) X_i and
    Z = sum_i exp(s_i); the host sums Z and divides once.

The attention MLP runs on X^T (features on partitions, streamed once,
16.8MB/core):
  PE : H^T[m,i] = sum_c W1c^T @ X^T_c  (8 matmuls/group of 512 rows)
  ACT: th = tanh(H^T + b1) -> bf16

The per-bag weighted sums need a free-axis weighted reduction, and no
single engine can absorb all of it: DVE's fused scalar_tensor_tensor
has no 2x uop (1 elem/partition/cycle = 68us for the whole tensor), PE
is busy with H, and the Pool engine rejects every arithmetic opcode.
So the work is SPLIT by block (bag == 2048-row block):

  xt-blocks (first 6): scores via a column-replicated W2 stationary --
      s_bcast[j,i] = sum_m w2rep[p,j] th[p,i] is identical across
      output partitions j, i.e. broadcast for free.  ACT exps it into
      a replicated wsave; DVE does the fused (X^T_c * wsave) multiply +
      accum_out reduction, one instruction per (block, chunk).
  xb-blocks (last _XB_COUNT=2): a second natural-layout copy of those
      rows streams in (+2MB each); scores stay in column form (1-col
      matmuls, ~27ns amortized -- LDWEIGHTS hides under the H stream),
      and U[b] accumulates on PE as 1-col matmuls with the X tile
      stationary, exactly 16 per group, into a per-bag PSUM column.

Measured (min of repeated runs; the device shows ~5-15% run-to-run
drift): 92.8us vs 110.9us for the session-start two-copy baseline.  PE is the pacing engine (~71us busy: H 55 + scores + xb-U
1-col matmuls); DVE ~53, ACT ~60, DMA 20.8MB; remaining time is ~7us
fixed NEFF preamble + first-data latency and a ~6us drain/epilogue.
The PE p-state ramp (0.65 -> 2.4GHz over ~3us of continuous work) is
pre-burned with a chain of short dummy matmuls that spans until the
first data lands (any PE idle resets the clock to 1.2GHz for ~3us),
and the scalar engine's lazy ACT_TABLE_LOAD is triggered early the
same way.  PSUM bank budget (8): hp 2x2 + scores 2 (shared between
the replicated and column forms -- single-buffering them stalls PE on
a cross-engine WAR against ACT's exp read) + xb-U accumulator 1.

Row permutation: xb tiles use the {16p+q} row grouping (16KB DMA runs
per partition); the HOST permutes those blocks' X^T columns to match
(col 512h+128j+p -> row 16p+4h+j), so the column-form scores line up
with the xb stationary partitions.  Bag sums are order-free within a
block and the softmax is global, so nothing else changes.

Scheduling notes from failed experiments (do not re-try blindly):
per-m-chunk tanh emission for ALL groups and reordering the startup
weight/quarter dma_starts both regressed ~8us (DMA queue assignment is
order-sensitive); the per-m tanh split is applied only to the final
group, where it shortens the drain chain without touching the steady
state.
"""

import numpy as np
import ml_dtypes

N_CORES = 8
F = 512  # feature dim
HID = 256  # hidden dim
P = 128  # partitions
BLK = 2048  # rows per DMA block (= bag size on the device path)
GR = 512  # rows per processing group
FC = F // P  # 4 feature chunks
MC = HID // P  # 2 hidden chunks
JT = GR // P  # 4 row-tiles per group

_COMPILED_CACHE = {}

# number of trailing blocks whose weighted sums run on PE from a second
# natural-layout copy of X (keeps DVE's work front-loaded so the kernel
# tail is pure PE).  0 = pure single-copy/DVE design.
_XB_COUNT = 2


def _xb_blocks(n_blocks):
    n_xb = min(_XB_COUNT, max(n_blocks - 2, 0))
    return set(range(n_blocks - n_xb, n_blocks))


def _group_stt_blocks(n_blocks):
    """xt-blocks whose DVE reductions run per-group instead of
    per-block.  With PE-side xb-blocks at the end, DVE has plenty of
    slack, so everything runs in cheap per-block form; without them the
    early blocks and the drain block go per-group so DVE starts early
    and finishes fast."""
    xb = _xb_blocks(n_blocks)
    base = set() if xb else {0, 1, 2, 3, 4, n_blocks - 1}
    return (base & set(range(n_blocks))) - xb


def _build_program(n_tiles):
    import concourse.bacc as bacc
    import concourse.mybir as mybir
    from concourse.tile import TileContext

    f32 = mybir.dt.float32
    bf16 = mybir.dt.bfloat16
    rows = n_tiles * P
    n_groups = rows // GR
    n_blocks = rows // BLK
    GPB = BLK // GR  # groups per block
    QPB = BLK // P  # 16 row-tiles per block
    LOOKAHEAD = 3
    xb_blocks = _xb_blocks(n_blocks)
    group_stt = _group_stt_blocks(n_blocks)
    n_xb = len(xb_blocks)
    xb_base = n_blocks - n_xb  # xb dram block k <-> global block xb_base+k
    block_mode = [
        b for b in range(n_blocks)
        if b not in xb_blocks and b not in group_stt
    ]
    # u slots 0..early_end-1 are final once the last block-mode DVE
    # reduction has run; the rest go out in the drain.
    early_end = (max(block_mode) * GPB + 1) if block_mode else 0

    nc = bacc.Bacc(
        "TRN2", target_bir_lowering=False, debug=False, num_devices=N_CORES
    )

    xt = nc.declare_dram_parameter("xt", [F, rows], bf16, isOutput=False)
    xb = nc.declare_dram_parameter(
        "xb", [max(n_xb, 1) * BLK, F], bf16, isOutput=False
    )
    w1 = nc.declare_dram_parameter("w1", [P, FC, MC, P], bf16, isOutput=False)
    b1 = nc.declare_dram_parameter("b1", [P, MC], f32, isOutput=False)
    w2r = nc.declare_dram_parameter("w2r", [P, MC, P], bf16, isOutput=False)
    w2c = nc.declare_dram_parameter("w2c", [P, MC, 1], bf16, isOutput=False)
    u_out = nc.declare_dram_parameter("u", [P, FC, n_groups], f32, isOutput=True)
    z_out = nc.declare_dram_parameter("z", [P, n_groups], f32, isOutput=True)

    with TileContext(nc) as tc:
        with (
            tc.tile_pool(name="const", bufs=1) as const_pool,
            tc.tile_pool(name="xt", bufs=5) as xt_pool,
            tc.tile_pool(name="xb", bufs=3) as xb_pool,
            tc.tile_pool(name="th", bufs=3) as th_pool,
            tc.tile_pool(name="yv", bufs=2) as yv_pool,
            tc.tile_pool(name="yg", bufs=2) as yg_pool,
            # PSUM budget (8 x 2KB banks): hp 2x2 + sp 2 + spc 1 + uacc 1.
            # sp needs 2: with 1, s(g)'s start=True waits on ACT's
            # exp(g-1) finishing its read -- a cross-engine WAR chain
            # that stalls PE every group.  uacc=1 is safe: consecutive
            # bags' accumulations are already serialized by the copy.
            tc.tile_pool(name="hp", bufs=2, space="PSUM") as hp_pool,
            tc.tile_pool(name="sp", bufs=2, space="PSUM") as sp_pool,
            tc.tile_pool(name="spc", bufs=1, space="PSUM") as spc_pool,
            tc.tile_pool(name="uacc", bufs=1, space="PSUM") as uacc_pool,
        ):
            xt_hist = {}
            xb_hist = {}

            # ---- startup: issue order is the critical path ----
            # w1b[p, c, m, j] = W1[c*128+p, m*128+j]
            w1b = const_pool.tile([P, FC, MC, P], bf16)
            nc.sync.dma_start(out=w1b[:, :, 0, :], in_=w1[:, :, 0, :])

            # block 0's first quarter right after w1b-m0, so H(0) can
            # start while the remaining startup DMAs are still issuing.
            xtt0 = xt_pool.tile([P, FC, BLK], bf16, name="xt", tag="xt")
            xt_hist[0] = xtt0
            nc.sync.dma_start(
                out=xtt0[:, 0:2, 0:GR],
                in_=xt[0 : 2 * P, 0:GR].rearrange("(c p) i -> p c i", p=P),
            )
            nc.sync.dma_start(
                out=xtt0[:, 2:4, 0:GR],
                in_=xt[2 * P :, 0:GR].rearrange("(c p) i -> p c i", p=P),
            )

            nc.sync.dma_start(
                out=xtt0[:, :, GR : 2 * GR],
                in_=xt[:, GR : 2 * GR].rearrange("(c p) i -> p c i", p=P),
            )
            nc.sync.dma_start(out=w1b[:, :, 1, :], in_=w1[:, :, 1, :])
            b1s = const_pool.tile([P, MC], f32)
            nc.sync.dma_start(out=b1s, in_=b1[:, :])
            # w2b[p, m, j] = W2[m*128+p] for every j (column-replicated)
            w2b = const_pool.tile([P, MC, P], bf16)
            nc.sync.dma_start(out=w2b, in_=w2r[:, :, :])
            # w2s[p, m, 0] = W2[m*128+p] (column form)
            w2s = const_pool.tile([P, MC, 1], bf16)
            nc.sync.dma_start(out=w2s, in_=w2c[:, :, :])

            for h in range(2, GPB):
                nc.sync.dma_start(
                    out=xtt0[:, :, h * GR : (h + 1) * GR],
                    in_=xt[:, h * GR : (h + 1) * GR].rearrange(
                        "(c p) i -> p c i", p=P
                    ),
                )

            def emit_load(bb):
                xtt = xt_pool.tile([P, FC, BLK], bf16, name="xt", tag="xt")
                xt_hist[bb] = xtt
                nc.sync.dma_start(
                    out=xtt,
                    in_=xt[:, bb * BLK : (bb + 1) * BLK].rearrange(
                        "(c p) i -> p c i", p=P
                    ),
                )
                if bb in xb_blocks:
                    xbt = xb_pool.tile(
                        [P, QPB, F], bf16, name="xb", tag="xb"
                    )
                    xb_hist[bb] = xbt
                    k = bb - xb_base
                    nc.sync.dma_start(
                        out=xbt,
                        in_=xb[k * BLK : (k + 1) * BLK, :].rearrange(
                            "(p q) f -> p q f", p=P
                        ),
                    )

            for bb in range(1, min(LOOKAHEAD + 1, n_blocks)):
                emit_load(bb)

            # exp(scores) for xt-blocks, partition-replicated.
            wsave = const_pool.tile([P, n_groups * GR], bf16)
            # exp(scores) for xb-blocks, column form [p, g*JT+j].
            wcol = const_pool.tile([P, n_groups * JT], bf16)
            # weighted-sum partials, indexed by group (see host side).
            u_sb = const_pool.tile([P, FC, n_groups], f32)
            # softmax-denominator partials: xt-groups are replicated
            # (host takes row 0); xb-groups are per-partition partials
            # (host sums the column).
            z_sb = const_pool.tile([P, n_groups], f32)

            # PE p-state warmup: the tensor engine ramps 0.65 -> 2.4 GHz
            # over ~3us of continuous execution.  A dependency-free chain
            # of dummy matmuls on zeroed SBUF (result never read) burns
            # the ramp while the first DMAs are still in flight, so the
            # first real H matmuls run at full clock.
            warm = const_pool.tile([P, GR], bf16)
            nc.gpsimd.memset(warm, 0)
            wp = sp_pool.tile([P, GR], f32, name="wp", tag="sp")
            # short 256-col chain: spans the window until block 0's
            # data lands with fine granularity, so H(0) starts warm
            # almost immediately instead of idling (idle resets the
            # p-state to 1.2GHz for the next ~3us of matmuls).
            for k in range(24):
                nc.tensor.matmul(
                    wp[:, 0:256],
                    warm[:, 0:P],
                    warm[:, 0:256],
                    start=(k == 0),
                    stop=(k == 23),
                )
            # also trigger the scalar engine's lazy ACT_TABLE_LOAD
            # (~1.3us) now instead of blocking the first real tanh.
            warm_a = const_pool.tile([P, 4], bf16)
            nc.scalar.activation(
                warm_a, warm[:, 0:4], mybir.ActivationFunctionType.Tanh
            )

            th_hist = {}
            u_bag = {}

            def emit_s(gg):
                # xt path: s_bcast[j, i] = sum_m W2[m] th[m, i] for every
                # j -- the column-replicated stationary makes all 128
                # output partitions identical, i.e. scores pre-broadcast.
                th_g = th_hist.pop(gg)
                sp = sp_pool.tile([P, GR], f32, name="sp", tag="sp")
                for m in range(MC):
                    nc.tensor.matmul(
                        sp,
                        w2b[:, m, :],
                        th_g[:, m, :],
                        start=(m == 0),
                        stop=(m == MC - 1),
                    )
                nc.scalar.activation(
                    wsave[:, gg * GR : (gg + 1) * GR],
                    sp,
                    mybir.ActivationFunctionType.Exp,
                    accum_out=z_sb[:, gg : gg + 1],
                )

            def emit_s_col(gg):
                # xb path: s[128j+p] per tile j, 1-col matmuls with th
                # chunks stationary (LDWEIGHTS hides under the H stream).
                th_g = th_hist.pop(gg)
                spc = sp_pool.tile([P, GR], f32, name="sp", tag="sp")
                for j in range(JT):
                    for m in range(MC):
                        nc.tensor.matmul(
                            spc[:, j : j + 1],
                            th_g[:, m, j * P : (j + 1) * P],
                            w2s[:, m, :],
                            start=(j == 0 and m == 0),
                            stop=(j == JT - 1 and m == MC - 1),
                        )
                nc.scalar.activation(
                    wcol[:, gg * JT : (gg + 1) * JT],
                    spc[:, 0:JT],
                    mybir.ActivationFunctionType.Exp,
                    accum_out=z_sb[:, gg : gg + 1],
                )

            def emit_u_block(bb):
                # xt path, fused multiply + free-axis accumulate over a
                # whole block: u[p, c, 4bb] = sum_i X^T[c*128+p, i]*w[i]
                xtb = xt_hist.pop(bb)
                wsl = wsave[:, bb * BLK : (bb + 1) * BLK]
                for c in range(FC):
                    y = yv_pool.tile([P, BLK], bf16, name="y", tag="y")
                    nc.vector.scalar_tensor_tensor(
                        out=y,
                        in0=xtb[:, c, :],
                        scalar=1.0,
                        in1=wsl,
                        op0=mybir.AluOpType.mult,
                        op1=mybir.AluOpType.mult,
                        accum_out=u_sb[:, c, bb * GPB : bb * GPB + 1],
                    )

            def emit_u_group(gg):
                # xt path, per-group variant for an early DVE start.
                bb, h = divmod(gg, GPB)
                xtb = xt_hist[bb]
                wsl = wsave[:, gg * GR : (gg + 1) * GR]
                for c in range(FC):
                    y = yg_pool.tile([P, GR], bf16, name="yg", tag="yg")
                    nc.vector.scalar_tensor_tensor(
                        out=y,
                        in0=xtb[:, c, h * GR : (h + 1) * GR],
                        scalar=1.0,
                        in1=wsl,
                        op0=mybir.AluOpType.mult,
                        op1=mybir.AluOpType.mult,
                        accum_out=u_sb[:, c, gg : gg + 1],
                    )
                if h == GPB - 1:
                    del xt_hist[bb]

            def emit_u_pe(gg):
                # xb path: U^T[:, c] += X_tile^T @ w_col on PE, 1-col
                # matmuls with the X tile stationary; accumulates in a
                # per-bag PSUM column pair over the bag's 16 tiles.
                bb, h = divmod(gg, GPB)
                xbt = xb_hist[bb]
                if bb not in u_bag:
                    # full-bank tile: start=True pending-zeroes the whole
                    # 2KB PSUM bank, so consecutive bags' accumulators
                    # must not share one.
                    u_bag[bb] = uacc_pool.tile(
                        [P, GR], f32, name="ub", tag="ub"
                    )
                ub = u_bag[bb]
                for j in range(JT):
                    q = h * JT + j
                    for c in range(FC):
                        nc.tensor.matmul(
                            ub[:, c : c + 1],
                            xbt[:, q, c * P : (c + 1) * P],
                            wcol[:, gg * JT + j : gg * JT + j + 1],
                            start=(q == 0 and c == 0),
                            stop=(q == QPB - 1 and c == FC - 1),
                        )
                if h == GPB - 1:
                    # bag finished: copy the PSUM column set to the
                    # block's first group slot (DVE is idle here).
                    nc.vector.tensor_copy(
                        out=u_sb[:, :, bb * GPB],
                        in_=u_bag.pop(bb)[:, 0:FC],
                    )
                    del xb_hist[bb]
                    xt_hist.pop(bb, None)

            def dispatch_s(gg):
                if gg // GPB in xb_blocks:
                    emit_s_col(gg)
                else:
                    emit_s(gg)

            # ---- main loop over 512-row groups (software-pipelined) ----
            for g in range(n_groups):
                bb, h = divmod(g, GPB)
                if h == 0 and (bb + LOOKAHEAD) < n_blocks and (
                    bb + LOOKAHEAD
                ) not in xt_hist:
                    emit_load(bb + LOOKAHEAD)
                xtb = xt_hist[bb]

                hp = hp_pool.tile([P, MC, GR], f32, name="hp", tag="hp")
                th = th_pool.tile([P, MC, GR], bf16, name="th", tag="th")
                th_hist[g] = th
                last = g == n_groups - 1
                for m in range(MC):
                    for c in range(FC):
                        nc.tensor.matmul(
                            hp[:, m, :],
                            w1b[:, c, m, :],
                            xtb[:, c, h * GR : (h + 1) * GR],
                            start=(c == 0),
                            stop=(c == FC - 1),
                        )
                    if last:
                        # drain: tanh(m0) overlaps H(m1) so the tail
                        # chain tanh -> s -> exp -> U starts sooner
                        nc.scalar.activation(
                            th[:, m, :],
                            hp[:, m, :],
                            mybir.ActivationFunctionType.Tanh,
                            bias=b1s[:, m : m + 1],
                        )
                if not last:
                    for m in range(MC):
                        nc.scalar.activation(
                            th[:, m, :],
                            hp[:, m, :],
                            mybir.ActivationFunctionType.Tanh,
                            bias=b1s[:, m : m + 1],
                        )

                if g >= 1:
                    gp = g - 1
                    dispatch_s(gp)
                    pb, ph = divmod(gp, GPB)
                    if pb in xb_blocks:
                        pass  # U runs at lag 2 below
                    elif pb in group_stt:
                        emit_u_group(gp)
                    elif ph == GPB - 1:
                        emit_u_block(pb)
                        if pb * GPB + 1 == early_end:
                            # last block-mode DVE reduction done: stream
                            # the finished u slots out under the rest.
                            nc.sync.dma_start(
                                out=u_out[:, :, :early_end],
                                in_=u_sb[:, :, :early_end],
                            )
                    # xb-path U runs at lag 2 (its exp pipelines one
                    # group behind PE).
                    gu = g - 2
                    if gu >= 0 and gu // GPB in xb_blocks:
                        emit_u_pe(gu)

            # ---- drain ----
            dispatch_s(n_groups - 1)
            if early_end < (n_blocks - 1) * GPB:
                # everything but the last block's slot is final once
                # U(n-2's block) lands; stream it under the last U MMs
                nc.sync.dma_start(
                    out=u_out[:, :, early_end : (n_blocks - 1) * GPB],
                    in_=u_sb[:, :, early_end : (n_blocks - 1) * GPB],
                )
            for gu in (n_groups - 2, n_groups - 1):
                if gu // GPB in xb_blocks:
                    emit_u_pe(gu)
                elif gu == n_groups - 1:
                    if (gu // GPB) in group_stt:
                        emit_u_group(gu)
                    else:
                        emit_u_block(gu // GPB)

            tail_slot = max(early_end, (n_blocks - 1) * GPB)
            nc.sync.dma_start(
                out=u_out[:, :, tail_slot:], in_=u_sb[:, :, tail_slot:]
            )
            nc.sync.dma_start(out=z_out[:, :], in_=z_sb)

    nc.compile()
    return nc


def _host_xt(X_core, n_blocks, xb_blocks):
    """X^T bf16 [F, rows].  xb-blocks get their columns permuted to the
    {16p+q} row grouping used by the xb tiles: col 512h+128j+p of the
    block maps to row 16p+4h+j."""
    rows = X_core.shape[0]
    xt = np.ascontiguousarray(X_core.T).astype(ml_dtypes.bfloat16)
    if xb_blocks:
        cols = np.arange(BLK)
        h, r = np.divmod(cols, GR)
        j, p = np.divmod(r, P)
        perm = 16 * p + 4 * h + j
        for bb in xb_blocks:
            blk = np.asarray(xt[:, bb * BLK : (bb + 1) * BLK])
            xt[:, bb * BLK : (bb + 1) * BLK] = blk[:, perm]
    return xt


def _run_device(X, W1, b1, W2, bag_rows, trace=False, trace_kwargs=None):
    from concourse.bass_utils import run_bass_kernel_spmd

    rows_per_core = X.shape[0] // N_CORES
    n_tiles = rows_per_core // P
    gpb = bag_rows // GR  # groups per bag
    n_bags_core = rows_per_core // bag_rows
    n_blocks = n_bags_core
    xb_blocks = _xb_blocks(n_blocks)
    group_stt = _group_stt_blocks(n_blocks)
    xb_base = n_blocks - len(xb_blocks)

    key = (rows_per_core, _XB_COUNT)
    if key in _COMPILED_CACHE:
        nc = _COMPILED_CACHE[key]
    else:
        nc = _build_program(n_tiles)
        _COMPILED_CACHE[key] = nc

    w1b = np.ascontiguousarray(
        np.asarray(W1, np.float32).reshape(FC, P, MC, P).transpose(1, 0, 2, 3)
    ).astype(ml_dtypes.bfloat16)
    w2col = np.ascontiguousarray(
        np.asarray(W2, np.float32).reshape(MC, P, 1).transpose(1, 0, 2)
    ).astype(ml_dtypes.bfloat16)
    w2rep = np.ascontiguousarray(
        np.broadcast_to(w2col, (P, MC, P))
    ).astype(ml_dtypes.bfloat16)
    b1s = np.ascontiguousarray(
        np.asarray(b1, np.float32).reshape(MC, P).T, np.float32
    )

    in_maps = []
    for c in range(N_CORES):
        xc = np.asarray(
            X[c * rows_per_core : (c + 1) * rows_per_core], np.float32
        )
        xt_c = _host_xt(xc, n_blocks, xb_blocks)
        if xb_blocks:
            xb_c = np.ascontiguousarray(
                xc[xb_base * BLK :]
            ).astype(ml_dtypes.bfloat16)
        else:
            xb_c = np.zeros((BLK, F), ml_dtypes.bfloat16)
        in_maps.append(
            {
                "xt": xt_c,
                "xb": xb_c,
                "w1": w1b,
                "b1": b1s,
                "w2r": w2rep,
                "w2c": w2col,
            }
        )
    kw = dict(trace_kwargs or {})
    res = run_bass_kernel_spmd(
        nc, in_maps, list(range(N_CORES)), trace=trace, **kw
    )

    U = np.zeros((N_CORES * n_bags_core, F), np.float32)
    Z = np.float64(0.0)
    for c in range(N_CORES):
        u = np.asarray(res.results[c]["u"], np.float32)  # [P, FC, n_groups]
        z = np.asarray(res.results[c]["z"], np.float64)  # [P, n_groups]
        cols = []
        for b in range(n_bags_core):
            if b in group_stt:
                cols.append(u[:, :, b * gpb : (b + 1) * gpb].sum(axis=2))
            else:
                cols.append(u[:, :, b * gpb])
        ub = np.stack(cols, axis=2)
        U[c * n_bags_core : (c + 1) * n_bags_core] = (
            ub.transpose(2, 1, 0).reshape(n_bags_core, F)
        )
        for g in range(z.shape[1]):
            if g // gpb in xb_blocks:
                Z += z[:, g].sum()
            else:
                Z += z[0, g]
    return U, Z, res


def _kernel_numpy(instance_features, bag_sizes, W1, b1, W2, b2):
    """Exact-math fallback for bag layouts the device program doesn't cover."""
    X = np.asarray(instance_features, np.float32)
    s = np.tanh(X @ W1 + b1) @ W2.reshape(-1, 1) + np.asarray(b2).reshape(1, -1)
    s = s - s.max()
    w = np.exp(s)
    w = w / w.sum()
    offsets = np.cumsum(np.asarray(bag_sizes, np.int64))
    seg = np.searchsorted(offsets, np.arange(X.shape[0]), side="right")
    out = np.zeros((len(bag_sizes), X.shape[1]), np.float32)
    np.add.at(out, seg[seg < len(bag_sizes)], (X * w)[seg < len(bag_sizes)])
    return out


def kernel(**inputs):
    X = np.asarray(inputs["instance_features"], np.float32)
    bag_sizes = np.asarray(inputs["bag_sizes"], np.int64)
    W1 = np.asarray(inputs["W1"], np.float32)
    b1 = np.asarray(inputs["b1"], np.float32)
    W2 = np.asarray(inputs["W2"], np.float32)
    b2 = np.asarray(inputs["b2"], np.float32)

    T, Fdim = X.shape
    B = bag_sizes.shape[0]
    bag = int(bag_sizes[0]) if B else 0
    # Device path constraints: equal whole bags per core, bag == BLK.
    aligned = (
        Fdim == F
        and B > 0
        and np.all(bag_sizes == bag)
        and bag == BLK
        and bag * B == T
        and T % N_CORES == 0
        and (T // N_CORES) % BLK == 0
        and (T // N_CORES) // BLK >= 3
    )
    if not aligned:
        return _kernel_numpy(X, bag_sizes, W1, b1, W2, b2)

    U, Z, _ = _run_device(X, W1, b1, W2, bag)
    return (U / np.float32(Z)).astype(np.float32)


# revision 58
# speedup vs baseline: 1.0283x; 1.0283x over previous
"""AttentionMILPooling Trainium2 kernel (hybrid xt/xb design).

Math (matches the jax reference):
    scores  = tanh(X @ W1 + b1) @ W2 + b2          # [T, 1]
    weights = softmax(scores, axis=0)              # global over all T
    out[b]  = sum_{i in bag b} weights[i] * X[i]   # [64, 512]

Identities:
  * b2 cancels in the softmax -> dropped.
  * |scores| <= sum|W2| ~ 13, exp fits fp32/bf16 range -> no max-subtract.
  * out[b] = U[b] / Z with U[b] = sum_{i in b} exp(s_i) X_i and
    Z = sum_i exp(s_i); the host sums Z and divides once.

The attention MLP runs on X^T (features on partitions, streamed once,
16.8MB/core):
  PE : H^T[m,i] = sum_c W1c^T @ X^T_c  (8 matmuls/group of 512 rows)
  ACT: th = tanh(H^T + b1) -> bf16

The per-bag weighted sums need a free-axis weighted reduction, and no
single engine can absorb all of it: DVE's fused scalar_tensor_tensor
has no 2x uop (1 elem/partition/cycle = 68us for the whole tensor), PE
is busy with H, and the Pool engine rejects every arithmetic opcode.
So the work is SPLIT by block (bag == 2048-row block):

  xt-blocks (first 6): scores via a column-replicated W2 stationary --
      s_bcast[j,i] = sum_m w2rep[p,j] th[p,i] is identical across
      output partitions j, i.e. broadcast for free.  ACT exps it into
      a replicated wsave; DVE does the fused (X^T_c * wsave) multiply +
      accum_out reduction, one instruction per (block, chunk).
  xb-blocks (last _XB_COUNT=2): a second natural-layout copy of those
      rows streams in (+2MB each); scores stay in column form (1-col
      matmuls, ~27ns amortized -- LDWEIGHTS hides under the H stream),
      and U[b] accumulates on PE as 1-col matmuls with the X tile
      stationary, exactly 16 per group, into a per-bag PSUM column.

Measured (min of repeated runs; the device shows ~5-15% run-to-run
drift): 91.8us vs 110.9us for the session-start two-copy baseline.
PE is the pacing engine (~71us busy: H 55 + scores + xb-U 1-col
matmuls); DVE ~53, ACT ~60, DMA 20.8MB; remaining time is ~7us fixed
NEFF preamble + first-data latency and a ~6us drain/epilogue.  The
early xt blocks (0-2) load as per-quarter dma_starts: a whole-block
2MB load finishes behind the startup queue backlog, and the resulting
H stall resets the PE p-state (one such stall cost ~3us).
The PE p-state ramp (0.65 -> 2.4GHz over ~3us of continuous work) is
pre-burned with a chain of short dummy matmuls that spans until the
first data lands (any PE idle resets the clock to 1.2GHz for ~3us),
and the scalar engine's lazy ACT_TABLE_LOAD is triggered early the
same way.  PSUM bank budget (8): hp 2x2 + scores 2 (shared between
the replicated and column forms -- single-buffering them stalls PE on
a cross-engine WAR against ACT's exp read) + xb-U accumulator 1.

Row permutation: xb tiles use the {16p+q} row grouping (16KB DMA runs
per partition); the HOST permutes those blocks' X^T columns to match
(col 512h+128j+p -> row 16p+4h+j), so the column-form scores line up
with the xb stationary partitions.  Bag sums are order-free within a
block and the softmax is global, so nothing else changes.

Scheduling notes from failed experiments (do not re-try blindly):
per-m-chunk tanh emission for ALL groups and reordering the startup
weight/quarter dma_starts both regressed ~8us (DMA queue assignment is
order-sensitive); the per-m tanh split is applied only to the final
group, where it shortens the drain chain without touching the steady
state.
"""

import numpy as np
import ml_dtypes

N_CORES = 8
F = 512  # feature dim
HID = 256  # hidden dim
P = 128  # partitions
BLK = 2048  # rows per DMA block (= bag size on the device path)
GR = 512  # rows per processing group
FC = F // P  # 4 feature chunks
MC = HID // P  # 2 hidden chunks
JT = GR // P  # 4 row-tiles per group

_COMPILED_CACHE = {}

# number of trailing blocks whose weighted sums run on PE from a second
# natural-layout copy of X (keeps DVE's work front-loaded so the kernel
# tail is pure PE).  0 = pure single-copy/DVE design.
_XB_COUNT = 2


def _xb_blocks(n_blocks):
    n_xb = min(_XB_COUNT, max(n_blocks - 2, 0))
    return set(range(n_blocks - n_xb, n_blocks))


def _group_stt_blocks(n_blocks):
    """xt-blocks whose DVE reductions run per-group instead of
    per-block.  With PE-side xb-blocks at the end, DVE has plenty of
    slack, so everything runs in cheap per-block form; without them the
    early blocks and the drain block go per-group so DVE starts early
    and finishes fast."""
    xb = _xb_blocks(n_blocks)
    base = set() if xb else {0, 1, 2, 3, 4, n_blocks - 1}
    return (base & set(range(n_blocks))) - xb


def _build_program(n_tiles):
    import concourse.bacc as bacc
    import concourse.mybir as mybir
    from concourse.tile import TileContext

    f32 = mybir.dt.float32
    bf16 = mybir.dt.bfloat16
    rows = n_tiles * P
    n_groups = rows // GR
    n_blocks = rows // BLK
    GPB = BLK // GR  # groups per block
    QPB = BLK // P  # 16 row-tiles per block
    LOOKAHEAD = 3
    xb_blocks = _xb_blocks(n_blocks)
    group_stt = _group_stt_blocks(n_blocks)
    n_xb = len(xb_blocks)
    xb_base = n_blocks - n_xb  # xb dram block k <-> global block xb_base+k
    block_mode = [
        b for b in range(n_blocks)
        if b not in xb_blocks and b not in group_stt
    ]
    # u slots 0..early_end-1 are final once the last block-mode DVE
    # reduction has run; the rest go out in the drain.
    early_end = (max(block_mode) * GPB + 1) if block_mode else 0

    nc = bacc.Bacc(
        "TRN2", target_bir_lowering=False, debug=False, num_devices=N_CORES
    )

    xt = nc.declare_dram_parameter("xt", [F, rows], bf16, isOutput=False)
    xb = nc.declare_dram_parameter(
        "xb", [max(n_xb, 1) * BLK, F], bf16, isOutput=False
    )
    w1 = nc.declare_dram_parameter("w1", [P, FC, MC, P], bf16, isOutput=False)
    b1 = nc.declare_dram_parameter("b1", [P, MC], f32, isOutput=False)
    w2r = nc.declare_dram_parameter("w2r", [P, MC, P], bf16, isOutput=False)
    w2c = nc.declare_dram_parameter("w2c", [P, MC, 1], bf16, isOutput=False)
    u_out = nc.declare_dram_parameter("u", [P, FC, n_groups], f32, isOutput=True)
    z_out = nc.declare_dram_parameter("z", [P, n_groups], f32, isOutput=True)

    with TileContext(nc) as tc:
        with (
            tc.tile_pool(name="const", bufs=1) as const_pool,
            tc.tile_pool(name="xt", bufs=5) as xt_pool,
            tc.tile_pool(name="xb", bufs=3) as xb_pool,
            tc.tile_pool(name="th", bufs=3) as th_pool,
            tc.tile_pool(name="yv", bufs=2) as yv_pool,
            tc.tile_pool(name="yg", bufs=2) as yg_pool,
            # PSUM budget (8 x 2KB banks): hp 2x2 + sp 2 + spc 1 + uacc 1.
            # sp needs 2: with 1, s(g)'s start=True waits on ACT's
            # exp(g-1) finishing its read -- a cross-engine WAR chain
            # that stalls PE every group.  uacc=1 is safe: consecutive
            # bags' accumulations are already serialized by the copy.
            tc.tile_pool(name="hp", bufs=2, space="PSUM") as hp_pool,
            tc.tile_pool(name="sp", bufs=2, space="PSUM") as sp_pool,
            tc.tile_pool(name="spc", bufs=1, space="PSUM") as spc_pool,
            tc.tile_pool(name="uacc", bufs=1, space="PSUM") as uacc_pool,
        ):
            xt_hist = {}
            xb_hist = {}

            # ---- startup: issue order is the critical path ----
            # w1b[p, c, m, j] = W1[c*128+p, m*128+j]
            w1b = const_pool.tile([P, FC, MC, P], bf16)
            nc.sync.dma_start(out=w1b[:, :, 0, :], in_=w1[:, :, 0, :])

            # block 0's first quarter right after w1b-m0, so H(0) can
            # start while the remaining startup DMAs are still issuing.
            xtt0 = xt_pool.tile([P, FC, BLK], bf16, name="xt", tag="xt")
            xt_hist[0] = xtt0
            nc.sync.dma_start(
                out=xtt0[:, 0:2, 0:GR],
                in_=xt[0 : 2 * P, 0:GR].rearrange("(c p) i -> p c i", p=P),
            )
            nc.sync.dma_start(
                out=xtt0[:, 2:4, 0:GR],
                in_=xt[2 * P :, 0:GR].rearrange("(c p) i -> p c i", p=P),
            )

            nc.sync.dma_start(
                out=xtt0[:, :, GR : 2 * GR],
                in_=xt[:, GR : 2 * GR].rearrange("(c p) i -> p c i", p=P),
            )
            nc.sync.dma_start(out=w1b[:, :, 1, :], in_=w1[:, :, 1, :])
            b1s = const_pool.tile([P, MC], f32)
            nc.sync.dma_start(out=b1s, in_=b1[:, :])
            # w2b[p, m, j] = W2[m*128+p] for every j (column-replicated)
            w2b = const_pool.tile([P, MC, P], bf16)
            nc.sync.dma_start(out=w2b, in_=w2r[:, :, :])
            # w2s[p, m, 0] = W2[m*128+p] (column form)
            w2s = const_pool.tile([P, MC, 1], bf16)
            nc.sync.dma_start(out=w2s, in_=w2c[:, :, :])

            for h in range(2, GPB):
                nc.sync.dma_start(
                    out=xtt0[:, :, h * GR : (h + 1) * GR],
                    in_=xt[:, h * GR : (h + 1) * GR].rearrange(
                        "(c p) i -> p c i", p=P
                    ),
                )

            def emit_load(bb, quarters=False):
                xtt = xt_pool.tile([P, FC, BLK], bf16, name="xt", tag="xt")
                xt_hist[bb] = xtt
                if quarters:
                    # early blocks: per-quarter dma_starts so each H
                    # group gates on its own quarter instead of the
                    # whole 2MB block (whose completion is delayed by
                    # queue backlog at startup -- a late block stalls
                    # PE and resets its p-state to 1.2GHz for ~3us).
                    for h in range(GPB):
                        nc.sync.dma_start(
                            out=xtt[:, :, h * GR : (h + 1) * GR],
                            in_=xt[
                                :, bb * BLK + h * GR : bb * BLK + (h + 1) * GR
                            ].rearrange("(c p) i -> p c i", p=P),
                        )
                else:
                    nc.sync.dma_start(
                        out=xtt,
                        in_=xt[:, bb * BLK : (bb + 1) * BLK].rearrange(
                            "(c p) i -> p c i", p=P
                        ),
                    )
                if bb in xb_blocks:
                    xbt = xb_pool.tile(
                        [P, QPB, F], bf16, name="xb", tag="xb"
                    )
                    xb_hist[bb] = xbt
                    k = bb - xb_base
                    nc.sync.dma_start(
                        out=xbt,
                        in_=xb[k * BLK : (k + 1) * BLK, :].rearrange(
                            "(p q) f -> p q f", p=P
                        ),
                    )

            for bb in range(1, min(LOOKAHEAD + 1, n_blocks)):
                emit_load(bb, quarters=(bb <= 2))

            # exp(scores) for xt-blocks, partition-replicated.
            wsave = const_pool.tile([P, n_groups * GR], bf16)
            # exp(scores) for xb-blocks, column form [p, g*JT+j].
            wcol = const_pool.tile([P, n_groups * JT], bf16)
            # weighted-sum partials, indexed by group (see host side).
            u_sb = const_pool.tile([P, FC, n_groups], f32)
            # softmax-denominator partials: xt-groups are replicated
            # (host takes row 0); xb-groups are per-partition partials
            # (host sums the column).
            z_sb = const_pool.tile([P, n_groups], f32)

            # PE p-state warmup: the tensor engine ramps 0.65 -> 2.4 GHz
            # over ~3us of continuous execution.  A dependency-free chain
            # of dummy matmuls on zeroed SBUF (result never read) burns
            # the ramp while the first DMAs are still in flight, so the
            # first real H matmuls run at full clock.
            warm = const_pool.tile([P, GR], bf16)
            nc.gpsimd.memset(warm, 0)
            wp = sp_pool.tile([P, GR], f32, name="wp", tag="sp")
            # short 256-col chain: spans the window until block 0's
            # data lands with fine granularity, so H(0) starts warm
            # almost immediately instead of idling (idle resets the
            # p-state to 1.2GHz for the next ~3us of matmuls).
            for k in range(24):
                nc.tensor.matmul(
                    wp[:, 0:256],
                    warm[:, 0:P],
                    warm[:, 0:256],
                    start=(k == 0),
                    stop=(k == 23),
                )
            # also trigger the scalar engine's lazy ACT_TABLE_LOAD
            # (~1.3us) now instead of blocking the first real tanh.
            warm_a = const_pool.tile([P, 4], bf16)
            nc.scalar.activation(
                warm_a, warm[:, 0:4], mybir.ActivationFunctionType.Tanh
            )

            th_hist = {}
            u_bag = {}

            def emit_s(gg):
                # xt path: s_bcast[j, i] = sum_m W2[m] th[m, i] for every
                # j -- the column-replicated stationary makes all 128
                # output partitions identical, i.e. scores pre-broadcast.
                th_g = th_hist.pop(gg)
                sp = sp_pool.tile([P, GR], f32, name="sp", tag="sp")
                for m in range(MC):
                    nc.tensor.matmul(
                        sp,
                        w2b[:, m, :],
                        th_g[:, m, :],
                        start=(m == 0),
                        stop=(m == MC - 1),
                    )
                nc.scalar.activation(
                    wsave[:, gg * GR : (gg + 1) * GR],
                    sp,
                    mybir.ActivationFunctionType.Exp,
                    accum_out=z_sb[:, gg : gg + 1],
                )

            def emit_s_col(gg):
                # xb path: s[128j+p] per tile j, 1-col matmuls with th
                # chunks stationary (LDWEIGHTS hides under the H stream).
                th_g = th_hist.pop(gg)
                spc = sp_pool.tile([P, GR], f32, name="sp", tag="sp")
                for j in range(JT):
                    for m in range(MC):
                        nc.tensor.matmul(
                            spc[:, j : j + 1],
                            th_g[:, m, j * P : (j + 1) * P],
                            w2s[:, m, :],
                            start=(j == 0 and m == 0),
                            stop=(j == JT - 1 and m == MC - 1),
                        )
                nc.scalar.activation(
                    wcol[:, gg * JT : (gg + 1) * JT],
                    spc[:, 0:JT],
                    mybir.ActivationFunctionType.Exp,
                    accum_out=z_sb[:, gg : gg + 1],
                )

            def emit_u_block(bb):
                # xt path, fused multiply + free-axis accumulate over a
                # whole block: u[p, c, 4bb] = sum_i X^T[c*128+p, i]*w[i]
                xtb = xt_hist.pop(bb)
                wsl = wsave[:, bb * BLK : (bb + 1) * BLK]
                for c in range(FC):
                    y = yv_pool.tile([P, BLK], bf16, name="y", tag="y")
                    nc.vector.scalar_tensor_tensor(
                        out=y,
                        in0=xtb[:, c, :],
                        scalar=1.0,
                        in1=wsl,
                        op0=mybir.AluOpType.mult,
                        op1=mybir.AluOpType.mult,
                        accum_out=u_sb[:, c, bb * GPB : bb * GPB + 1],
                    )

            def emit_u_group(gg):
                # xt path, per-group variant for an early DVE start.
                bb, h = divmod(gg, GPB)
                xtb = xt_hist[bb]
                wsl = wsave[:, gg * GR : (gg + 1) * GR]
                for c in range(FC):
                    y = yg_pool.tile([P, GR], bf16, name="yg", tag="yg")
                    nc.vector.scalar_tensor_tensor(
                        out=y,
                        in0=xtb[:, c, h * GR : (h + 1) * GR],
                        scalar=1.0,
                        in1=wsl,
                        op0=mybir.AluOpType.mult,
                        op1=mybir.AluOpType.mult,
                        accum_out=u_sb[:, c, gg : gg + 1],
                    )
                if h == GPB - 1:
                    del xt_hist[bb]

            def emit_u_pe(gg):
                # xb path: U^T[:, c] += X_tile^T @ w_col on PE, 1-col
                # matmuls with the X tile stationary; accumulates in a
                # per-bag PSUM column pair over the bag's 16 tiles.
                bb, h = divmod(gg, GPB)
                xbt = xb_hist[bb]
                if bb not in u_bag:
                    # full-bank tile: start=True pending-zeroes the whole
                    # 2KB PSUM bank, so consecutive bags' accumulators
                    # must not share one.
                    u_bag[bb] = uacc_pool.tile(
                        [P, GR], f32, name="ub", tag="ub"
                    )
                ub = u_bag[bb]
                for j in range(JT):
                    q = h * JT + j
                    for c in range(FC):
                        nc.tensor.matmul(
                            ub[:, c : c + 1],
                            xbt[:, q, c * P : (c + 1) * P],
                            wcol[:, gg * JT + j : gg * JT + j + 1],
                            start=(q == 0 and c == 0),
                            stop=(q == QPB - 1 and c == FC - 1),
                        )
                if h == GPB - 1:
                    # bag finished: copy the PSUM column set to the
                    # block's first group slot (DVE is idle here).
                    nc.vector.tensor_copy(
                        out=u_sb[:, :, bb * GPB],
                        in_=u_bag.pop(bb)[:, 0:FC],
                    )
                    del xb_hist[bb]
                    xt_hist.pop(bb, None)

            def dispatch_s(gg):
                if gg // GPB in xb_blocks:
                    emit_s_col(gg)
                else:
                    emit_s(gg)

            # ---- main loop over 512-row groups (software-pipelined) ----
            for g in range(n_groups):
                bb, h = divmod(g, GPB)
                if h == 0 and (bb + LOOKAHEAD) < n_blocks and (
                    bb + LOOKAHEAD
                ) not in xt_hist:
                    emit_load(bb + LOOKAHEAD)
                xtb = xt_hist[bb]

                hp = hp_pool.tile([P, MC, GR], f32, name="hp", tag="hp")
                th = th_pool.tile([P, MC, GR], bf16, name="th", tag="th")
                th_hist[g] = th
                last = g == n_groups - 1
                for m in range(MC):
                    for c in range(FC):
                        nc.tensor.matmul(
                            hp[:, m, :],
                            w1b[:, c, m, :],
                            xtb[:, c, h * GR : (h + 1) * GR],
                            start=(c == 0),
                            stop=(c == FC - 1),
                        )
                    if last:
                        # drain: tanh(m0) overlaps H(m1) so the tail
                        # chain tanh -> s -> exp -> U starts sooner
                        nc.scalar.activation(
                            th[:, m, :],
                            hp[:, m, :],
                            mybir.ActivationFunctionType.Tanh,
                            bias=b1s[:, m : m + 1],
                        )
                if not last:
                    for m in range(MC):
                        nc.scalar.activation(
                            th[:, m, :],
                            hp[:, m, :],
                            mybir.ActivationFunctionType.Tanh,
                            bias=b1s[:, m : m + 1],
                        )

                if g >= 1:
                    gp = g - 1
                    dispatch_s(gp)
                    pb, ph = divmod(gp, GPB)
                    if pb in xb_blocks:
                        pass  # U runs at lag 2 below
                    elif pb in group_stt:
                        emit_u_group(gp)
                    elif ph == GPB - 1:
                        emit_u_block(pb)
                        if pb * GPB + 1 == early_end:
                            # last block-mode DVE reduction done: stream
                            # the finished u slots out under the rest.
                            nc.sync.dma_start(
                                out=u_out[:, :, :early_end],
                                in_=u_sb[:, :, :early_end],
                            )
                    # xb-path U runs at lag 2 (its exp pipelines one
                    # group behind PE).
                    gu = g - 2
                    if gu >= 0 and gu // GPB in xb_blocks:
                        emit_u_pe(gu)

            # ---- drain ----
            dispatch_s(n_groups - 1)
            if early_end < (n_blocks - 1) * GPB:
                # everything but the last block's slot is final once
                # U(n-2's block) lands; stream it under the last U MMs
                nc.sync.dma_start(
                    out=u_out[:, :, early_end : (n_blocks - 1) * GPB],
                    in_=u_sb[:, :, early_end : (n_blocks - 1) * GPB],
                )
            for gu in (n_groups - 2, n_groups - 1):
                if gu // GPB in xb_blocks:
                    emit_u_pe(gu)
                elif gu == n_groups - 1:
                    if (gu // GPB) in group_stt:
                        emit_u_group(gu)
                    else:
                        emit_u_block(gu // GPB)

            tail_slot = max(early_end, (n_blocks - 1) * GPB)
            nc.sync.dma_start(
                out=u_out[:, :, tail_slot:], in_=u_sb[:, :, tail_slot:]
            )
            nc.sync.dma_start(out=z_out[:, :], in_=z_sb)

    nc.compile()
    return nc


def _host_xt(X_core, n_blocks, xb_blocks):
    """X^T bf16 [F, rows].  xb-blocks get their columns permuted to the
    {16p+q} row grouping used by the xb tiles: col 512h+128j+p of the
    block maps to row 16p+4h+j."""
    rows = X_core.shape[0]
    xt = np.ascontiguousarray(X_core.T).astype(ml_dtypes.bfloat16)
    if xb_blocks:
        cols = np.arange(BLK)
        h, r = np.divmod(cols, GR)
        j, p = np.divmod(r, P)
        perm = 16 * p + 4 * h + j
        for bb in xb_blocks:
            blk = np.asarray(xt[:, bb * BLK : (bb + 1) * BLK])
            xt[:, bb * BLK : (bb + 1) * BLK] = blk[:, perm]
    return xt


def _run_device(X, W1, b1, W2, bag_rows, trace=False, trace_kwargs=None):
    from concourse.bass_utils import run_bass_kernel_spmd

    rows_per_core = X.shape[0] // N_CORES
    n_tiles = rows_per_core // P
    gpb = bag_rows // GR  # groups per bag
    n_bags_core = rows_per_core // bag_rows
    n_blocks = n_bags_core
    xb_blocks = _xb_blocks(n_blocks)
    group_stt = _group_stt_blocks(n_blocks)
    xb_base = n_blocks - len(xb_blocks)

    key = (rows_per_core, _XB_COUNT)
    if key in _COMPILED_CACHE:
        nc = _COMPILED_CACHE[key]
    else:
        nc = _build_program(n_tiles)
        _COMPILED_CACHE[key] = nc

    w1b = np.ascontiguousarray(
        np.asarray(W1, np.float32).reshape(FC, P, MC, P).transpose(1, 0, 2, 3)
    ).astype(ml_dtypes.bfloat16)
    w2col = np.ascontiguousarray(
        np.asarray(W2, np.float32).reshape(MC, P, 1).transpose(1, 0, 2)
    ).astype(ml_dtypes.bfloat16)
    w2rep = np.ascontiguousarray(
        np.broadcast_to(w2col, (P, MC, P))
    ).astype(ml_dtypes.bfloat16)
    b1s = np.ascontiguousarray(
        np.asarray(b1, np.float32).reshape(MC, P).T, np.float32
    )

    in_maps = []
    for c in range(N_CORES):
        xc = np.asarray(
            X[c * rows_per_core : (c + 1) * rows_per_core], np.float32
        )
        xt_c = _host_xt(xc, n_blocks, xb_blocks)
        if xb_blocks:
            xb_c = np.ascontiguousarray(
                xc[xb_base * BLK :]
            ).astype(ml_dtypes.bfloat16)
        else:
            xb_c = np.zeros((BLK, F), ml_dtypes.bfloat16)
        in_maps.append(
            {
                "xt": xt_c,
                "xb": xb_c,
                "w1": w1b,
                "b1": b1s,
                "w2r": w2rep,
                "w2c": w2col,
            }
        )
    kw = dict(trace_kwargs or {})
    res = run_bass_kernel_spmd(
        nc, in_maps, list(range(N_CORES)), trace=trace, **kw
    )

    U = np.zeros((N_CORES * n_bags_core, F), np.float32)
    Z = np.float64(0.0)
    for c in range(N_CORES):
        u = np.asarray(res.results[c]["u"], np.float32)  # [P, FC, n_groups]
        z = np.asarray(res.results[c]["z"], np.float64)  # [P, n_groups]
        cols = []
        for b in range(n_bags_core):
            if b in group_stt:
                cols.append(u[:, :, b * gpb : (b + 1) * gpb].sum(axis=2))
            else:
                cols.append(u[:, :, b * gpb])
        ub = np.stack(cols, axis=2)
        U[c * n_bags_core : (c + 1) * n_bags_core] = (
            ub.transpose(2, 1, 0).reshape(n_bags_core, F)
        )
        for g in range(z.shape[1]):
            if g // gpb in xb_blocks:
                Z += z[:, g].sum()
            else:
                Z += z[0, g]
    return U, Z, res


def _kernel_numpy(instance_features, bag_sizes, W1, b1, W2, b2):
    """Exact-math fallback for bag layouts the device program doesn't cover."""
    X = np.asarray(instance_features, np.float32)
    s = np.tanh(X @ W1 + b1) @ W2.reshape(-1, 1) + np.asarray(b2).reshape(1, -1)
    s = s - s.max()
    w = np.exp(s)
    w = w / w.sum()
    offsets = np.cumsum(np.asarray(bag_sizes, np.int64))
    seg = np.searchsorted(offsets, np.arange(X.shape[0]), side="right")
    out = np.zeros((len(bag_sizes), X.shape[1]), np.float32)
    np.add.at(out, seg[seg < len(bag_sizes)], (X * w)[seg < len(bag_sizes)])
    return out


def kernel(**inputs):
    X = np.asarray(inputs["instance_features"], np.float32)
    bag_sizes = np.asarray(inputs["bag_sizes"], np.int64)
    W1 = np.asarray(inputs["W1"], np.float32)
    b1 = np.asarray(inputs["b1"], np.float32)
    W2 = np.asarray(inputs["W2"], np.float32)
    b2 = np.asarray(inputs["b2"], np.float32)

    T, Fdim = X.shape
    B = bag_sizes.shape[0]
    bag = int(bag_sizes[0]) if B else 0
    # Device path constraints: equal whole bags per core, bag == BLK.
    aligned = (
        Fdim == F
        and B > 0
        and np.all(bag_sizes == bag)
        and bag == BLK
        and bag * B == T
        and T % N_CORES == 0
        and (T // N_CORES) % BLK == 0
        and (T // N_CORES) // BLK >= 3
    )
    if not aligned:
        return _kernel_numpy(X, bag_sizes, W1, b1, W2, b2)

    U, Z, _ = _run_device(X, W1, b1, W2, bag)
    return (U / np.float32(Z)).astype(np.float32)


# revision 59
# speedup vs baseline: 1.0433x; 1.0146x over previous
"""AttentionMILPooling Trainium2 kernel (hybrid xt/xb design).

Math (matches the jax reference):
    scores  = tanh(X @ W1 + b1) @ W2 + b2          # [T, 1]
    weights = softmax(scores, axis=0)              # global over all T
    out[b]  = sum_{i in bag b} weights[i] * X[i]   # [64, 512]

Identities:
  * b2 cancels in the softmax -> dropped.
  * |scores| <= sum|W2| ~ 13, exp fits fp32/bf16 range -> no max-subtract.
  * out[b] = U[b] / Z with U[b] = sum_{i in b} exp(s_i) X_i and
    Z = sum_i exp(s_i); the host sums Z and divides once.

The attention MLP runs on X^T (features on partitions, streamed once,
16.8MB/core):
  PE : H^T[m,i] = sum_c W1c^T @ X^T_c  (8 matmuls/group of 512 rows)
  ACT: th = tanh(H^T + b1) -> bf16

The per-bag weighted sums need a free-axis weighted reduction, and no
single engine can absorb all of it: DVE's fused scalar_tensor_tensor
has no 2x uop (1 elem/partition/cycle = 68us for the whole tensor), PE
is busy with H, and the Pool engine rejects every arithmetic opcode.
So the work is SPLIT by block (bag == 2048-row block):

  xt-blocks (first 6): scores via a column-replicated W2 stationary --
      s_bcast[j,i] = sum_m w2rep[p,j] th[p,i] is identical across
      output partitions j, i.e. broadcast for free.  ACT exps it into
      a replicated wsave; DVE does the fused (X^T_c * wsave) multiply +
      accum_out reduction, one instruction per (block, chunk).
  xb-blocks (last _XB_COUNT=2): a second natural-layout copy of those
      rows streams in (+2MB each); scores stay in column form (1-col
      matmuls, ~27ns amortized -- LDWEIGHTS hides under the H stream),
      and U[b] accumulates on PE as 1-col matmuls with the X tile
      stationary, exactly 16 per group, into a per-bag PSUM column.

Measured (min of repeated runs; the device shows ~5-15% run-to-run
drift): 91.1us vs 110.9us for the session-start two-copy baseline.
PE is the pacing engine (~71us busy: H 55 + scores + xb-U 1-col
matmuls); DVE ~53, ACT ~60, DMA 20.8MB; remaining time is ~7us fixed
NEFF preamble + first-data latency and a ~6us drain/epilogue.  The
early xt blocks (0-2) load as per-quarter dma_starts: a whole-block
2MB load finishes behind the startup queue backlog, and the resulting
H stall resets the PE p-state (one such stall cost ~3us).
The PE p-state ramp (0.65 -> 2.4GHz over ~3us of continuous work) is
pre-burned with a chain of short dummy matmuls that spans until the
first data lands (any PE idle resets the clock to 1.2GHz for ~3us),
and the scalar engine's lazy ACT_TABLE_LOAD is triggered early the
same way.  PSUM bank budget (8): hp 2x2 + scores 2 (shared between
the replicated and column forms -- single-buffering them stalls PE on
a cross-engine WAR against ACT's exp read) + xb-U accumulator 1.

Row permutation: xb tiles use the {16p+q} row grouping (16KB DMA runs
per partition); the HOST permutes those blocks' X^T columns to match
(col 512h+128j+p -> row 16p+4h+j), so the column-form scores line up
with the xb stationary partitions.  Bag sums are order-free within a
block and the softmax is global, so nothing else changes.

Scheduling notes from failed experiments (do not re-try blindly):
per-m-chunk tanh emission for ALL groups and reordering the startup
weight/quarter dma_starts both regressed ~8us (DMA queue assignment is
order-sensitive); the per-m tanh split is applied only to the final
group, where it shortens the drain chain without touching the steady
state.
"""

import numpy as np
import ml_dtypes

N_CORES = 8
F = 512  # feature dim
HID = 256  # hidden dim
P = 128  # partitions
BLK = 2048  # rows per DMA block (= bag size on the device path)
GR = 512  # rows per processing group
FC = F // P  # 4 feature chunks
MC = HID // P  # 2 hidden chunks
JT = GR // P  # 4 row-tiles per group

_COMPILED_CACHE = {}

# number of trailing blocks whose weighted sums run on PE from a second
# natural-layout copy of X (keeps DVE's work front-loaded so the kernel
# tail is pure PE).  0 = pure single-copy/DVE design.
_XB_COUNT = 2


def _xb_blocks(n_blocks):
    n_xb = min(_XB_COUNT, max(n_blocks - 2, 0))
    return set(range(n_blocks - n_xb, n_blocks))


def _group_stt_blocks(n_blocks):
    """xt-blocks whose DVE reductions run per-group instead of
    per-block.  With PE-side xb-blocks at the end, DVE has plenty of
    slack, so everything runs in cheap per-block form; without them the
    early blocks and the drain block go per-group so DVE starts early
    and finishes fast."""
    xb = _xb_blocks(n_blocks)
    base = set() if xb else {0, 1, 2, 3, 4, n_blocks - 1}
    return (base & set(range(n_blocks))) - xb


def _build_program(n_tiles):
    import concourse.bacc as bacc
    import concourse.mybir as mybir
    from concourse.tile import TileContext

    f32 = mybir.dt.float32
    bf16 = mybir.dt.bfloat16
    rows = n_tiles * P
    n_groups = rows // GR
    n_blocks = rows // BLK
    GPB = BLK // GR  # groups per block
    QPB = BLK // P  # 16 row-tiles per block
    LOOKAHEAD = 3
    xb_blocks = _xb_blocks(n_blocks)
    group_stt = _group_stt_blocks(n_blocks)
    n_xb = len(xb_blocks)
    xb_base = n_blocks - n_xb  # xb dram block k <-> global block xb_base+k
    block_mode = [
        b for b in range(n_blocks)
        if b not in xb_blocks and b not in group_stt
    ]
    # u slots 0..early_end-1 are final once the last block-mode DVE
    # reduction has run; the rest go out in the drain.
    early_end = (max(block_mode) * GPB + 1) if block_mode else 0

    nc = bacc.Bacc(
        "TRN2", target_bir_lowering=False, debug=False, num_devices=N_CORES
    )

    xt = nc.declare_dram_parameter("xt", [F, rows], bf16, isOutput=False)
    xb = nc.declare_dram_parameter(
        "xb", [max(n_xb, 1) * BLK, F], bf16, isOutput=False
    )
    w1 = nc.declare_dram_parameter("w1", [P, FC, MC, P], bf16, isOutput=False)
    b1 = nc.declare_dram_parameter("b1", [P, MC], f32, isOutput=False)
    w2r = nc.declare_dram_parameter("w2r", [P, MC, P], bf16, isOutput=False)
    w2c = nc.declare_dram_parameter("w2c", [P, MC, 1], bf16, isOutput=False)
    u_out = nc.declare_dram_parameter("u", [P, FC, n_groups], f32, isOutput=True)
    z_out = nc.declare_dram_parameter("z", [P, n_groups], f32, isOutput=True)

    with TileContext(nc) as tc:
        with (
            tc.tile_pool(name="const", bufs=1) as const_pool,
            tc.tile_pool(name="xt", bufs=5) as xt_pool,
            tc.tile_pool(name="xb", bufs=3) as xb_pool,
            tc.tile_pool(name="th", bufs=3) as th_pool,
            tc.tile_pool(name="yv", bufs=2) as yv_pool,
            tc.tile_pool(name="yg", bufs=2) as yg_pool,
            # PSUM budget (8 x 2KB banks): hp 2x2 + sp 2 + spc 1 + uacc 1.
            # sp needs 2: with 1, s(g)'s start=True waits on ACT's
            # exp(g-1) finishing its read -- a cross-engine WAR chain
            # that stalls PE every group.  uacc=1 is safe: consecutive
            # bags' accumulations are already serialized by the copy.
            tc.tile_pool(name="hp", bufs=2, space="PSUM") as hp_pool,
            tc.tile_pool(name="sp", bufs=2, space="PSUM") as sp_pool,
            tc.tile_pool(name="spc", bufs=1, space="PSUM") as spc_pool,
            tc.tile_pool(name="uacc", bufs=1, space="PSUM") as uacc_pool,
        ):
            xt_hist = {}
            xb_hist = {}

            # ---- startup: issue order is the critical path ----
            # w1b[p, c, m, j] = W1[c*128+p, m*128+j]
            w1b = const_pool.tile([P, FC, MC, P], bf16)
            nc.sync.dma_start(out=w1b[:, :, 0, :], in_=w1[:, :, 0, :])

            # block 0's first quarter right after w1b-m0, so H(0) can
            # start while the remaining startup DMAs are still issuing.
            xtt0 = xt_pool.tile([P, FC, BLK], bf16, name="xt", tag="xt")
            xt_hist[0] = xtt0
            nc.sync.dma_start(
                out=xtt0[:, 0:2, 0:GR],
                in_=xt[0 : 2 * P, 0:GR].rearrange("(c p) i -> p c i", p=P),
            )
            nc.sync.dma_start(
                out=xtt0[:, 2:4, 0:GR],
                in_=xt[2 * P :, 0:GR].rearrange("(c p) i -> p c i", p=P),
            )

            nc.sync.dma_start(
                out=xtt0[:, :, GR : 2 * GR],
                in_=xt[:, GR : 2 * GR].rearrange("(c p) i -> p c i", p=P),
            )
            nc.sync.dma_start(out=w1b[:, :, 1, :], in_=w1[:, :, 1, :])
            b1s = const_pool.tile([P, MC], f32)
            nc.sync.dma_start(out=b1s, in_=b1[:, :])
            # w2b[p, m, j] = W2[m*128+p] for every j (column-replicated)
            w2b = const_pool.tile([P, MC, P], bf16)
            nc.sync.dma_start(out=w2b, in_=w2r[:, :, :])
            # w2s[p, m, 0] = W2[m*128+p] (column form)
            w2s = const_pool.tile([P, MC, 1], bf16)
            nc.sync.dma_start(out=w2s, in_=w2c[:, :, :])

            for h in range(2, GPB):
                nc.sync.dma_start(
                    out=xtt0[:, :, h * GR : (h + 1) * GR],
                    in_=xt[:, h * GR : (h + 1) * GR].rearrange(
                        "(c p) i -> p c i", p=P
                    ),
                )

            def emit_load(bb, quarters=False):
                xtt = xt_pool.tile([P, FC, BLK], bf16, name="xt", tag="xt")
                xt_hist[bb] = xtt
                if quarters:
                    # early blocks: per-quarter dma_starts so each H
                    # group gates on its own quarter instead of the
                    # whole 2MB block (whose completion is delayed by
                    # queue backlog at startup -- a late block stalls
                    # PE and resets its p-state to 1.2GHz for ~3us).
                    for h in range(GPB):
                        nc.sync.dma_start(
                            out=xtt[:, :, h * GR : (h + 1) * GR],
                            in_=xt[
                                :, bb * BLK + h * GR : bb * BLK + (h + 1) * GR
                            ].rearrange("(c p) i -> p c i", p=P),
                        )
                else:
                    nc.sync.dma_start(
                        out=xtt,
                        in_=xt[:, bb * BLK : (bb + 1) * BLK].rearrange(
                            "(c p) i -> p c i", p=P
                        ),
                    )
                if bb in xb_blocks:
                    xbt = xb_pool.tile(
                        [P, QPB, F], bf16, name="xb", tag="xb"
                    )
                    xb_hist[bb] = xbt
                    k = bb - xb_base
                    nc.sync.dma_start(
                        out=xbt,
                        in_=xb[k * BLK : (k + 1) * BLK, :].rearrange(
                            "(p q) f -> p q f", p=P
                        ),
                    )

            for bb in range(1, min(LOOKAHEAD + 1, n_blocks)):
                emit_load(bb, quarters=(bb <= 2))

            # exp(scores) for xt-blocks, partition-replicated.
            wsave = const_pool.tile([P, n_groups * GR], bf16)
            # exp(scores) for xb-blocks, column form [p, g*JT+j].
            wcol = const_pool.tile([P, n_groups * JT], bf16)
            # weighted-sum partials, indexed by group (see host side).
            u_sb = const_pool.tile([P, FC, n_groups], f32)
            # softmax-denominator partials: xt-groups are replicated
            # (host takes row 0); xb-groups are per-partition partials
            # (host sums the column).
            z_sb = const_pool.tile([P, n_groups], f32)

            # PE p-state warmup: the tensor engine ramps 0.65 -> 2.4 GHz
            # over ~3us of continuous execution.  A dependency-free chain
            # of dummy matmuls on zeroed SBUF (result never read) burns
            # the ramp while the first DMAs are still in flight, so the
            # first real H matmuls run at full clock.
            warm = const_pool.tile([P, GR], bf16)
            nc.gpsimd.memset(warm, 0)
            wp = sp_pool.tile([P, GR], f32, name="wp", tag="sp")
            # short 256-col chain: spans the window until block 0's
            # data lands with fine granularity, so H(0) starts warm
            # almost immediately instead of idling (idle resets the
            # p-state to 1.2GHz for the next ~3us of matmuls).
            for k in range(24):
                nc.tensor.matmul(
                    wp[:, 0:256],
                    warm[:, 0:P],
                    warm[:, 0:256],
                    start=(k == 0),
                    stop=(k == 23),
                )
            # also trigger the scalar engine's lazy ACT_TABLE_LOAD
            # (~1.3us) now instead of blocking the first real tanh.
            warm_a = const_pool.tile([P, 4], bf16)
            nc.scalar.activation(
                warm_a, warm[:, 0:4], mybir.ActivationFunctionType.Tanh
            )

            th_hist = {}
            u_bag = {}

            def emit_s(gg):
                # xt path: s_bcast[j, i] = sum_m W2[m] th[m, i] for every
                # j -- the column-replicated stationary makes all 128
                # output partitions identical, i.e. scores pre-broadcast.
                th_g = th_hist.pop(gg)
                sp = sp_pool.tile([P, GR], f32, name="sp", tag="sp")
                for m in range(MC):
                    nc.tensor.matmul(
                        sp,
                        w2b[:, m, :],
                        th_g[:, m, :],
                        start=(m == 0),
                        stop=(m == MC - 1),
                    )
                nc.scalar.activation(
                    wsave[:, gg * GR : (gg + 1) * GR],
                    sp,
                    mybir.ActivationFunctionType.Exp,
                    accum_out=z_sb[:, gg : gg + 1],
                )

            def emit_s_col(gg):
                # xb path: s[128j+p] per tile j, 1-col matmuls with th
                # chunks stationary (LDWEIGHTS hides under the H stream).
                th_g = th_hist.pop(gg)
                spc = sp_pool.tile([P, GR], f32, name="sp", tag="sp")
                for j in range(JT):
                    for m in range(MC):
                        nc.tensor.matmul(
                            spc[:, j : j + 1],
                            th_g[:, m, j * P : (j + 1) * P],
                            w2s[:, m, :],
                            start=(j == 0 and m == 0),
                            stop=(j == JT - 1 and m == MC - 1),
                        )
                nc.scalar.activation(
                    wcol[:, gg * JT : (gg + 1) * JT],
                    spc[:, 0:JT],
                    mybir.ActivationFunctionType.Exp,
                    accum_out=z_sb[:, gg : gg + 1],
                )

            def emit_u_block(bb):
                # xt path, fused multiply + free-axis accumulate over a
                # whole block: u[p, c, 4bb] = sum_i X^T[c*128+p, i]*w[i]
                xtb = xt_hist.pop(bb)
                wsl = wsave[:, bb * BLK : (bb + 1) * BLK]
                for c in range(FC):
                    y = yv_pool.tile([P, BLK], bf16, name="y", tag="y")
                    nc.vector.scalar_tensor_tensor(
                        out=y,
                        in0=xtb[:, c, :],
                        scalar=1.0,
                        in1=wsl,
                        op0=mybir.AluOpType.mult,
                        op1=mybir.AluOpType.mult,
                        accum_out=u_sb[:, c, bb * GPB : bb * GPB + 1],
                    )

            def emit_u_group(gg):
                # xt path, per-group variant for an early DVE start.
                bb, h = divmod(gg, GPB)
                xtb = xt_hist[bb]
                wsl = wsave[:, gg * GR : (gg + 1) * GR]
                for c in range(FC):
                    y = yg_pool.tile([P, GR], bf16, name="yg", tag="yg")
                    nc.vector.scalar_tensor_tensor(
                        out=y,
                        in0=xtb[:, c, h * GR : (h + 1) * GR],
                        scalar=1.0,
                        in1=wsl,
                        op0=mybir.AluOpType.mult,
                        op1=mybir.AluOpType.mult,
                        accum_out=u_sb[:, c, gg : gg + 1],
                    )
                if h == GPB - 1:
                    del xt_hist[bb]

            def emit_u_pe(gg):
                # xb path: U^T[:, c] += X_tile^T @ w_col on PE, 1-col
                # matmuls with the X tile stationary; accumulates in a
                # per-bag PSUM column pair over the bag's 16 tiles.
                bb, h = divmod(gg, GPB)
                xbt = xb_hist[bb]
                if bb not in u_bag:
                    # full-bank tile: start=True pending-zeroes the whole
                    # 2KB PSUM bank, so consecutive bags' accumulators
                    # must not share one.
                    u_bag[bb] = uacc_pool.tile(
                        [P, GR], f32, name="ub", tag="ub"
                    )
                ub = u_bag[bb]
                for j in range(JT):
                    q = h * JT + j
                    for c in range(FC):
                        nc.tensor.matmul(
                            ub[:, c : c + 1],
                            xbt[:, q, c * P : (c + 1) * P],
                            wcol[:, gg * JT + j : gg * JT + j + 1],
                            start=(q == 0 and c == 0),
                            stop=(q == QPB - 1 and c == FC - 1),
                        )
                if h == GPB - 1:
                    # bag finished: copy the PSUM column set to the
                    # block's first group slot (DVE is idle here).
                    nc.vector.tensor_copy(
                        out=u_sb[:, :, bb * GPB],
                        in_=u_bag.pop(bb)[:, 0:FC],
                    )
                    del xb_hist[bb]
                    xt_hist.pop(bb, None)

            def dispatch_s(gg):
                if gg // GPB in xb_blocks:
                    emit_s_col(gg)
                else:
                    emit_s(gg)

            # ---- main loop over 512-row groups (software-pipelined) ----
            for g in range(n_groups):
                bb, h = divmod(g, GPB)
                if h == 0 and (bb + LOOKAHEAD) < n_blocks and (
                    bb + LOOKAHEAD
                ) not in xt_hist:
                    emit_load(bb + LOOKAHEAD)
                xtb = xt_hist[bb]

                hp = hp_pool.tile([P, MC, GR], f32, name="hp", tag="hp")
                th = th_pool.tile([P, MC, GR], bf16, name="th", tag="th")
                th_hist[g] = th
                last = g == n_groups - 1
                for m in range(MC):
                    for c in range(FC):
                        nc.tensor.matmul(
                            hp[:, m, :],
                            w1b[:, c, m, :],
                            xtb[:, c, h * GR : (h + 1) * GR],
                            start=(c == 0),
                            stop=(c == FC - 1),
                        )
                    if last:
                        # drain: tanh(m0) overlaps H(m1) so the tail
                        # chain tanh -> s -> exp -> U starts sooner
                        nc.scalar.activation(
                            th[:, m, :],
                            hp[:, m, :],
                            mybir.ActivationFunctionType.Tanh,
                            bias=b1s[:, m : m + 1],
                        )
                if not last:
                    for m in range(MC):
                        nc.scalar.activation(
                            th[:, m, :],
                            hp[:, m, :],
                            mybir.ActivationFunctionType.Tanh,
                            bias=b1s[:, m : m + 1],
                        )

                if g >= 1:
                    gp = g - 1
                    dispatch_s(gp)
                    pb, ph = divmod(gp, GPB)
                    if pb in xb_blocks:
                        pass  # U runs at lag 2 below
                    elif pb in group_stt:
                        emit_u_group(gp)
                    elif ph == GPB - 1:
                        emit_u_block(pb)
                        if pb * GPB + 1 == early_end:
                            # last block-mode DVE reduction done: stream
                            # the finished u slots out under the rest.
                            nc.sync.dma_start(
                                out=u_out[:, :, :early_end],
                                in_=u_sb[:, :, :early_end],
                            )
                    # xb-path U runs at lag 2 (its exp pipelines one
                    # group behind PE).
                    gu = g - 2
                    if gu >= 0 and gu // GPB in xb_blocks:
                        emit_u_pe(gu)

            # ---- drain ----
            dispatch_s(n_groups - 1)
            if early_end < (n_blocks - 1) * GPB:
                # everything but the last block's slot is final once
                # U(n-2's block) lands; stream it under the last U MMs
                nc.sync.dma_start(
                    out=u_out[:, :, early_end : (n_blocks - 1) * GPB],
                    in_=u_sb[:, :, early_end : (n_blocks - 1) * GPB],
                )
            for gu in (n_groups - 2, n_groups - 1):
                if gu // GPB in xb_blocks:
                    emit_u_pe(gu)
                elif gu == n_groups - 1:
                    if (gu // GPB) in group_stt:
                        emit_u_group(gu)
                    else:
                        emit_u_block(gu // GPB)

            tail_slot = max(early_end, (n_blocks - 1) * GPB)
            nc.sync.dma_start(
                out=u_out[:, :, tail_slot:], in_=u_sb[:, :, tail_slot:]
            )
            nc.sync.dma_start(out=z_out[:, :], in_=z_sb)

    nc.compile()
    return nc


def _host_xt(X_core, n_blocks, xb_blocks):
    """X^T bf16 [F, rows].  xb-blocks get their columns permuted to the
    {16p+q} row grouping used by the xb tiles: col 512h+128j+p of the
    block maps to row 16p+4h+j."""
    rows = X_core.shape[0]
    xt = np.ascontiguousarray(X_core.T).astype(ml_dtypes.bfloat16)
    if xb_blocks:
        cols = np.arange(BLK)
        h, r = np.divmod(cols, GR)
        j, p = np.divmod(r, P)
        perm = 16 * p + 4 * h + j
        for bb in xb_blocks:
            blk = np.asarray(xt[:, bb * BLK : (bb + 1) * BLK])
            xt[:, bb * BLK : (bb + 1) * BLK] = blk[:, perm]
    return xt


def _run_device(X, W1, b1, W2, bag_rows, trace=False, trace_kwargs=None):
    from concourse.bass_utils import run_bass_kernel_spmd

    rows_per_core = X.shape[0] // N_CORES
    n_tiles = rows_per_core // P
    gpb = bag_rows // GR  # groups per bag
    n_bags_core = rows_per_core // bag_rows
    n_blocks = n_bags_core
    xb_blocks = _xb_blocks(n_blocks)
    group_stt = _group_stt_blocks(n_blocks)
    xb_base = n_blocks - len(xb_blocks)

    key = (rows_per_core, _XB_COUNT)
    if key in _COMPILED_CACHE:
        nc = _COMPILED_CACHE[key]
    else:
        nc = _build_program(n_tiles)
        _COMPILED_CACHE[key] = nc

    w1b = np.ascontiguousarray(
        np.asarray(W1, np.float32).reshape(FC, P, MC, P).transpose(1, 0, 2, 3)
    ).astype(ml_dtypes.bfloat16)
    w2col = np.ascontiguousarray(
        np.asarray(W2, np.float32).reshape(MC, P, 1).transpose(1, 0, 2)
    ).astype(ml_dtypes.bfloat16)
    w2rep = np.ascontiguousarray(
        np.broadcast_to(w2col, (P, MC, P))
    ).astype(ml_dtypes.bfloat16)
    b1s = np.ascontiguousarray(
        np.asarray(b1, np.float32).reshape(MC, P).T, np.float32
    )

    in_maps = []
    for c in range(N_CORES):
        xc = np.asarray(
            X[c * rows_per_core : (c + 1) * rows_per_core], np.float32
        )
        xt_c = _host_xt(xc, n_blocks, xb_blocks)
        if xb_blocks:
            xb_c = np.ascontiguousarray(
                xc[xb_base * BLK :]
            ).astype(ml_dtypes.bfloat16)
        else:
            xb_c = np.zeros((BLK, F), ml_dtypes.bfloat16)
        in_maps.append(
            {
                "xt": xt_c,
                "xb": xb_c,
                "w1": w1b,
                "b1": b1s,
                "w2r": w2rep,
                "w2c": w2col,
            }
        )
    kw = dict(trace_kwargs or {})
    res = run_bass_kernel_spmd(
        nc, in_maps, list(range(N_CORES)), trace=trace, **kw
    )

    U = np.zeros((N_CORES * n_bags_core, F), np.float32)
    Z = np.float64(0.0)
    for c in range(N_CORES):
        u = np.asarray(res.results[c]["u"], np.float32)  # [P, FC, n_groups]
        z = np.asarray(res.results[c]["z"], np.float64)  # [P, n_groups]
        cols = []
        for b in range(n_bags_core):
            if b in group_stt:
                cols.append(u[:, :, b * gpb : (b + 1) * gpb].sum(axis=2))
            else:
                cols.append(u[:, :, b * gpb])
        ub = np.stack(cols, axis=2)
        U[c * n_bags_core : (c + 1) * n_bags_core] = (
            ub.transpose(2, 1, 0).reshape(n_bags_core, F)
        )
        for g in range(z.shape[1]):
            if g // gpb in xb_blocks:
                Z += z[:, g].sum()
            else:
                Z += z[0, g]
    return U, Z, res


def _kernel_numpy(instance_features, bag_sizes, W1, b1, W2, b2):
    """Exact-math fallback for bag layouts the device program doesn't cover."""
    X = np.asarray(instance_features, np.float32)
    s = np.tanh(X @ W1 + b1) @ W2.reshape(-1, 1) + np.asarray(b2).reshape(1, -1)
    s = s - s.max()
    w = np.exp(s)
    w = w / w.sum()
    offsets = np.cumsum(np.asarray(bag_sizes, np.int64))
    seg = np.searchsorted(offsets, np.arange(X.shape[0]), side="right")
    out = np.zeros((len(bag_sizes), X.shape[1]), np.float32)
    np.add.at(out, seg[seg < len(bag_sizes)], (X * w)[seg < len(bag_sizes)])
    return out


def kernel(**inputs):
    X = np.asarray(inputs["instance_features"], np.float32)
    bag_sizes = np.asarray(inputs["bag_sizes"], np.int64)
    W1 = np.asarray(inputs["W1"], np.float32)
    b1 = np.asarray(inputs["b1"], np.float32)
    W2 = np.asarray(inputs["W2"], np.float32)
    b2 = np.asarray(inputs["b2"], np.float32)

    T, Fdim = X.shape
    B = bag_sizes.shape[0]
    bag = int(bag_sizes[0]) if B else 0
    # Device path constraints: equal whole bags per core, bag == BLK.
    aligned = (
        Fdim == F
        and B > 0
        and np.all(bag_sizes == bag)
        and bag == BLK
        and bag * B == T
        and T % N_CORES == 0
        and (T // N_CORES) % BLK == 0
        and (T // N_CORES) // BLK >= 3
    )
    if not aligned:
        return _kernel_numpy(X, bag_sizes, W1, b1, W2, b2)

    U, Z, _ = _run_device(X, W1, b1, W2, bag)
    return (U / np.float32(Z)).astype(np.float32)
